# revision 65
# baseline (speedup 1.0000x reference)
"""Trainium2 Bass kernel for channel-attention (XCA-style) nn.Module.

Per batch (8 batches -> 8 NeuronCores, pure data parallel):
  qkv = w_qkv @ x            (1x1 conv, 192 -> 576 channels)
  qkv = dwconv3x3(qkv)       (depthwise, per-channel 3x3, zero pad)
  q,k,v = split(qkv); per head (4 heads, 48 ch):
  score = softmax((q/||q||) @ (k/||k||)^T * temp)   contracting hw=16384
  out   = w_proj @ (score @ v)

v2 design notes (per core):
 - qkv matmul on PE in bf16, streamed in chunks of 16 image rows
   (+1 halo row each side, recomputed), whole-chunk x loads.
 - qkv psum evicted (ACT, f32->bf16) into ring with row stride 132
   (128 cols + 4 zero gap): depthwise taps are shifted APs with correct
   zero padding.  ring B = ring A shifted +1 elem (SBUF->SBUF DMA)
   keeps dx=+-1 taps 4-byte aligned.
 - q,k depthwise 3x3 on DVE (8 TS muls 4x + 1 ACT mul + 8 TT adds 2x).
 - v depthwise 3x3 on PE: per tap, matmul with diag(w_tap) as lhsT and
   the shifted ring view as rhs, accumulated over 9 taps in PSUM.
   Convolved v stays in SBUF (no DRAM spill round trip).
 - q,k chunks transposed on PE; score^T accumulated in a single packed
   [96,2,96] PSUM bank across all 128 chunklets.
 - L2 norms via ACT Square+accum_out; temperature and 1/||.|| folded
   into score; full-row softmax with -1e30 cross-head mask.
 - stage B folds M = w_proj @ probs (2+1 small matmuls); stage C is
   y = M @ v: 4 matmuls per 1024-px tile straight from SBUF v.
"""

import sys

sys.path.insert(0, "/opt/trn_rl_repo")

import numpy as np
import ml_dtypes

import concourse.bass as bass
import concourse.mybir as mybir
import concourse.tile as tile
from concourse import bacc
from concourse.bass import ts, ds
from concourse.bass_utils import run_bass_kernel_spmd
from concourse.masks import make_identity

F32 = mybir.dt.float32
BF16 = mybir.dt.bfloat16
FP8 = mybir.dt.float8e4
WQ_SCALE = 32.0  # host-side scale on wq to avoid fp8 subnormals

DIM = 192
NH = 4
CH = DIM // NH  # 48
C3 = 3 * DIM  # 576
H = 128
W = 128
HW = H * W
B = 8

NPB = 5  # qkv channel partition blocks: 4x128 + 64
PB_SZ = [128, 128, 128, 128, 64]
CHUNK = 16  # image rows per chunk
NCHUNK = H // CHUNK
RROWS = CHUNK + 2  # ring rows = chunk + halo
RSTR = 132  # ring row stride in elements
NTPC = (W * CHUNK) // 128  # 128-px chunklets per chunk

MUL = mybir.AluOpType.mult
ADD = mybir.AluOpType.add
AF = mybir.ActivationFunctionType
AX = mybir.AxisListType

# dx=0 taps first: they read only ring A, buying time for the ring-B copy
TAPS = [(-1, 0), (0, 0), (1, 0), (-1, -1), (-1, 1), (0, -1), (0, 1), (1, -1), (1, 1)]


def wix(dy, dx):
    return (dy + 1) * 3 + (dx + 1)


def build():
    nc = bacc.Bacc(None, target_bir_lowering=False)

    xd = nc.dram_tensor("x", [DIM, HW], BF16, kind="ExternalInput")
    wqd = nc.dram_tensor("wq", [128, 2, C3], BF16, kind="ExternalInput")
    wpd = nc.dram_tensor("wp", [96, 2, DIM], BF16, kind="ExternalInput")
    dwd = nc.dram_tensor("dww", [128, 4, 9], F32, kind="ExternalInput")
    # diag matrices for v depthwise on PE: [:,i,0:128]=diag(wv pb3 tap i),
    # [0:64,i,128:192]=diag(wv pb4 tap i)
    dvd = nc.dram_tensor("dvv", [128, 9, DIM], BF16, kind="ExternalInput")
    tvd = nc.dram_tensor("tmpv", [128, 3], F32, kind="ExternalInput")
    mkd = nc.dram_tensor("mask", [96, 96], F32, kind="ExternalInput")
    isd = nc.dram_tensor("idshift", [128, 64], BF16, kind="ExternalInput")
    outd = nc.dram_tensor("out", [DIM, HW], F32, kind="ExternalOutput")
    # v spill scratch (bf16)
    vda = nc.dram_tensor("vsb3d", [128, HW], BF16, kind="Internal")
    vdb = nc.dram_tensor("vsb4d", [64, HW], BF16, kind="Internal")

    with tile.TileContext(nc) as tc:
        _body(nc, tc, xd, wqd, wpd, dwd, dvd, tvd, mkd, isd, outd, vda, vdb)
    nc.compile()
    return nc


def _body(nc, tc, xd, wqd, wpd, dwd, dvd, tvd, mkd, isd, outd, vda, vdb):
    import contextlib

    xr = xd

    with contextlib.ExitStack() as ctx:
        consts = ctx.enter_context(tc.tile_pool(name="consts", bufs=1))
        smx = ctx.enter_context(tc.tile_pool(name="smx", bufs=1))

        # ---------------- constants ----------------
        wq = consts.tile([128, 2, C3], BF16, tag="wq")
        nc.sync.dma_start(wq[:], wqd[:, :, :])
        wp = consts.tile([96, 2, DIM], BF16, tag="wp")
        nc.sync.dma_start(wp[:], wpd[:, :, :])
        dww = consts.tile([128, 4, 9], F32, tag="dww")
        nc.sync.dma_start(dww[:], dwd[:, :, :])
        dvv = consts.tile([128, 9, DIM], BF16, tag="dvv")
        nc.sync.dma_start(dvv[:], dvd[:, :, :])
        tmpv = consts.tile([128, 3], F32, tag="tmpv")
        nc.sync.dma_start(tmpv[:], tvd[:, :])
        mask = consts.tile([96, 96], F32, tag="mask")
        nc.sync.dma_start(mask[:], mkd[:, :])
        ident = consts.tile([128, 128], F32, tag="ident")
        make_identity(nc, ident[:])
        identb = consts.tile([128, 128], BF16, tag="identb")
        make_identity(nc, identb[:])
        idsh = consts.tile([128, 64], BF16, tag="idsh")
        nc.sync.dma_start(idsh[:], isd[:, :])
        n2 = consts.tile([128, 3], F32, tag="n2")
        nc.vector.memset(n2[:], 0.0)

        # ============ stage A: qkv + dw + norms + score^T ============
        with contextlib.ExitStack() as sa:
            ringa = sa.enter_context(tc.tile_pool(name="ringa", bufs=2))
            ringb = sa.enter_context(tc.tile_pool(name="ringb", bufs=2))
            xp = sa.enter_context(tc.tile_pool(name="xp", bufs=1))
            vst = sa.enter_context(tc.tile_pool(name="vst", bufs=2))
            pssc = sa.enter_context(
                tc.tile_pool(name="pssc", bufs=1, space=bass.MemorySpace.PSUM)
            )
            sb = contextlib.ExitStack()
            psqkv = sb.enter_context(
                tc.tile_pool(name="psqkv", bufs=2, space=bass.MemorySpace.PSUM)
            )
            tpsp = sb.enter_context(
                tc.tile_pool(name="tps", bufs=1, space=bass.MemorySpace.PSUM)
            )
            psvdw = sb.enter_context(
                tc.tile_pool(name="psvdw", bufs=2, space=bass.MemorySpace.PSUM)
            )
            dwt = sa.enter_context(tc.tile_pool(name="dwt", bufs=1))
            qkp = sa.enter_context(tc.tile_pool(name="qkp", bufs=2))
            qtp = sa.enter_context(tc.tile_pool(name="qtp", bufs=1))
            nrm = sa.enter_context(tc.tile_pool(name="nrm", bufs=2))

            scps = [
                pssc.tile([96, 96], F32, tag=f"sc{i}", name=f"scps{i}")
                for i in range(2)
            ]



            rings = {}

            def emit_front(c):
                row_lo = 1 if c == 0 else 0
                row_hi = RROWS - 1 if c == NCHUNK - 1 else RROWS
                npix = (row_hi - row_lo) * W
                base_px = (c * CHUNK - 1 + row_lo) * W

                # merged ring tiles: pblocks 0..3 in rAm, pblock 4 in rA4
                rAm = ringa.tile([128, 4, RROWS, RSTR], BF16, tag="rAm", name=f"rAm_{c}")
                rA4 = ringa.tile([64, RROWS, RSTR], BF16, tag="rA4", name=f"rA4_{c}")
                rBm = ringb.tile([128, 4, RROWS, RSTR], BF16, tag="rBm", name=f"rBm_{c}")
                rB4 = ringb.tile([64, RROWS, RSTR], BF16, tag="rB4", name=f"rB4_{c}")
                rings[c] = (rAm, rA4, rBm, rB4)
                # zero the gap columns (stale from slot reuse)
                nc.vector.memset(rAm[:, :, :, 128:132], 0.0)
                nc.vector.memset(rA4[:, :, 128:132], 0.0)
                if c == 0:
                    nc.vector.memset(rAm[:, :, 0, :], 0.0)
                    nc.vector.memset(rA4[:, 0, :], 0.0)
                if c == NCHUNK - 1:
                    nc.vector.memset(rAm[:, :, RROWS - 1, :], 0.0)
                    nc.vector.memset(rA4[:, RROWS - 1, :], 0.0)

                # --- x in (whole chunk) + qkv matmul + evict to ring A ---
                xt = xp.tile([128, 2, RROWS * W], BF16, tag="xt")
                nc.sync.dma_start(xt[:, 0, :npix], xr[0:128, ds(base_px, npix)])
                nc.sync.dma_start(
                    xt[0:64, 1, :npix], xr[128:192, ds(base_px, npix)]
                )
                nt = (npix + 511) // 512
                nel = RROWS * RSTR
                for mb in range(NPB):
                    msz = PB_SZ[mb]
                    for j in range(nt):
                        w0 = j * 512
                        wn = min(512, npix - w0)
                        ps = psqkv.tile([128, 512], F32, tag="qkvps")
                        nc.tensor.matmul(
                            ps[:msz, :wn],
                            wq[:, 0, ds(mb * 128, msz)],
                            xt[:, 0, ds(w0, wn)],
                            start=True,
                            stop=False,
                        )
                        nc.tensor.matmul(
                            ps[:msz, :wn],
                            wq[0:64, 1, ds(mb * 128, msz)],
                            xt[0:64, 1, ds(w0, wn)],
                            start=False,
                            stop=True,
                        )
                        rr = row_lo + (w0 // 128)
                        nr = wn // 128
                        if mb < 4:
                            dst = rAm[:msz, mb, rr : rr + nr, 0:128]
                        else:
                            dst = rA4[:, rr : rr + nr, 0:128]
                        src = ps[:msz, :wn].rearrange("p (r w) -> p r w", w=128)
                        nc.scalar.copy(dst, src)
                    # ring B copy for this pblock right after its evictions
                    if mb < 4:
                        av = rAm[:, mb, :, :].rearrange("p r s -> p (r s)")
                        bv = rBm[:, mb, :, :].rearrange("p r s -> p (r s)")
                    else:
                        av = rA4[:].rearrange("p r s -> p (r s)")
                        bv = rB4[:].rearrange("p r s -> p (r s)")
                    nc.sync.dma_start(bv[:, 1:nel], av[:, 0 : nel - 1])
                nc.vector.memset(rBm[:, :, 0, 0:1], 0.0)
                nc.vector.memset(rB4[:, 0, 0:1], 0.0)

            def emit_back(c):
                rAm, rA4, rBm, rB4 = rings.pop(c)

                def rA(pb):
                    return rAm[:, pb, :, :] if pb < 4 else rA4

                def rB(pb):
                    return rBm[:, pb, :, :] if pb < 4 else rB4

                def tap(pb, dy, dx, rlo, rn):
                    # shifted view covering output rows [rlo, rlo+rn)
                    if dx == 0:
                        return rA(pb)[: PB_SZ[pb], 1 + dy + rlo : 1 + dy + rlo + rn, 0:128]
                    return rB(pb)[
                        : PB_SZ[pb],
                        1 + dy + rlo : 1 + dy + rlo + rn,
                        1 + dx : 129 + dx,
                    ]

                # --- v pblock3 depthwise on PE: diag-matmul accumulation ---
                for j in range(4):  # 512-px output tiles
                    vp3 = psvdw.tile([128, 512], F32, tag="vps", name=f"vp3_{c}_{j}")
                    for i, (dy, dx) in enumerate(TAPS):
                        nc.tensor.matmul(
                            vp3[:, :],
                            dvv[:, wix(dy, dx), 0:128],
                            tap(3, dy, dx, 4 * j, 4),
                            start=(i == 0),
                            stop=(i == 8),
                        )
                    csl = ds(c * CHUNK * W + j * 512, 512)
                    va = vst.tile([128, 512], BF16, tag="va")
                    nc.scalar.copy(va[:], vp3[:, :])
                    nc.sync.dma_start(vda[:, csl], va[:])

                # --- q,k + v pblock4 depthwise 3x3 on DVE ---
                qk = qkp.tile([128, 3, CHUNK * W], BF16, tag="qk")
                vb = vst.tile([64, CHUNK * W], BF16, tag="vb", bufs=1)
                for pb in (0, 1, 2, 4):
                    psz = PB_SZ[pb]
                    if pb < 3:
                        dest = qk[:psz, pb, :]
                    else:
                        dest = vb[:, :]
                    dest3 = dest.rearrange("p (r w) -> p r w", w=128)
                    widx = pb if pb < 3 else 3
                    prev = None
                    for i, (dy, dx) in enumerate(TAPS):
                        wsc = dww[:psz, widx, wix(dy, dx) : wix(dy, dx) + 1]
                        m = dwt.tile([128, CHUNK, W], BF16, tag=f"m{i % 2}")
                        nc.vector.tensor_scalar(
                            m[:psz], tap(pb, dy, dx, 0, CHUNK), wsc, None, op0=MUL
                        )
                        if i == 0:
                            prev = m
                            continue
                        last = i == 8
                        o3 = (
                            dest3
                            if last
                            else dwt.tile([128, CHUNK, W], BF16, tag=f"a{i % 2}")
                        )
                        oap = o3 if last else o3[:psz]
                        nc.vector.tensor_tensor(oap, prev[:psz], m[:psz], op=ADD)
                        if not last:
                            prev = o3

                # --- spill v pblock4 chunk to DRAM ---
                nc.sync.dma_start(vdb[:, ds(c * CHUNK * W, CHUNK * W)], vb[:])

                # --- norms (q,k pblocks 0..2) ---
                for pb in range(3):
                    sq = dwt.tile([128, CHUNK, W], BF16, tag="sq", bufs=1)
                    part = nrm.tile([128, 1], F32, tag="part")
                    nc.scalar.activation(
                        sq[:].rearrange("p r w -> p (r w)"),
                        qk[:, pb, :],
                        AF.Square,
                        accum_out=part[:],
                    )
                    nc.vector.tensor_tensor(
                        n2[:, pb : pb + 1], n2[:, pb : pb + 1], part[:], op=ADD
                    )

                # --- transpose q,k + score matmuls ---
                qt = qtp.tile([128, NTPC, DIM], BF16, tag="qt")
                kt = qtp.tile([128, NTPC, DIM], BF16, tag="kt")
                for ii in range(0, NTPC, 2):
                    tq = tpsp.tile([128, 2, 192], BF16, tag="tq")
                    tk = tpsp.tile([128, 2, 192], BF16, tag="tk")
                    for u in range(2):
                        i = ii + u
                        nc.tensor.transpose(
                            tq[:, u, 0:128], qk[:, 0, ts(i, 128)], identb[:]
                        )
                        nc.tensor.transpose(
                            tq[:, u, 128:192],
                            qk[0:64, 1, ts(i, 128)],
                            identb[0:64, 0:64],
                        )
                        nc.tensor.transpose(
                            tk[:, u, 0:64],
                            qk[64:128, 1, ts(i, 128)],
                            idsh[64:128, :],
                        )
                        nc.tensor.transpose(
                            tk[:, u, 64:192], qk[:, 2, ts(i, 128)], identb[:]
                        )
                    nc.scalar.copy(qt[:, ii : ii + 2, :], tq[:])
                    nc.scalar.copy(kt[:, ii : ii + 2, :], tk[:])
                for i in range(NTPC):
                    first = c == 0 and i == 0
                    last = c == NCHUNK - 1 and i == NTPC - 1
                    nc.tensor.matmul(
                        scps[0][:],
                        kt[:, i, 0:96],
                        qt[:, i, 0:96],
                        start=first,
                        stop=last,
                    )
                    nc.tensor.matmul(
                        scps[1][:],
                        kt[:, i, 96:192],
                        qt[:, i, 96:192],
                        start=first,
                        stop=last,
                    )

            # software-pipelined emission: chunk c+1's front-end (x, qkv,
            # evictions, ring-B) is emitted before chunk c's back-end so the
            # DVE never waits for ring data at chunk boundaries
            emit_front(0)
            for c in range(NCHUNK):
                if c + 1 < NCHUNK:
                    emit_front(c + 1)
                emit_back(c)

            sb.close()
            psB = sa.enter_context(
                tc.tile_pool(name="psB", bufs=1, space=bass.MemorySpace.PSUM)
            )
            # ---------- score finalize + softmax ----------
            rs = smx.tile([128, 3], F32, tag="rs")
            nc.scalar.activation(rs[:], n2[:], AF.Sqrt)
            nc.vector.tensor_scalar(
                rs[:], rs[:], 1e-12, None, op0=mybir.AluOpType.max
            )
            nc.vector.reciprocal(rs[:], rs[:])
            nc.vector.tensor_tensor(rs[:], rs[:], tmpv[:], op=MUL)

            # partition-aligned scale vectors for score rows
            rsq_b = smx.tile([96, 1], F32, tag="rsqb")
            rsk_a = smx.tile([96, 1], F32, tag="rska")
            rsk_b = smx.tile([96, 1], F32, tag="rskb")
            nc.sync.dma_start(rsq_b[0:32, :], rs[96:128, 0:1])
            nc.sync.dma_start(rsq_b[32:96, :], rs[0:64, 1:2])
            nc.sync.dma_start(rsk_a[0:64, :], rs[64:128, 1:2])
            nc.sync.dma_start(rsk_a[64:96, :], rs[0:32, 2:3])
            nc.sync.dma_start(rsk_b[:, :], rs[32:128, 2:3])
            rsq_a = rs[:, 0:1]

            sc_t = smx.tile([96, 2, 96], F32, tag="sct")
            nc.scalar.activation(
                sc_t[:, 0, :], scps[0][:], AF.Copy, scale=rsk_a[:]
            )
            nc.scalar.activation(
                sc_t[:, 1, :], scps[1][:], AF.Copy, scale=rsk_b[:]
            )
            scp2 = [psB.tile([96, 96], F32, tag=f"sc2_{i}", name=f"scp2_{i}") for i in range(2)]
            nc.tensor.transpose(scp2[0][:], sc_t[:, 0, :], ident[0:96, 0:96])
            nc.tensor.transpose(scp2[1][:], sc_t[:, 1, :], ident[0:96, 0:96])

            # evict full rows with q-scale, then add -1e30 off-diag mask so
            # the full-row softmax ignores cross-head blocks
            sc = smx.tile([96, 2, 96], F32, tag="sc")
            for g in range(2):
                qsc = rsq_a[0:96] if g == 0 else rsq_b[0:96]
                nc.scalar.activation(
                    sc[:, g, :], scp2[g][:], AF.Copy, scale=qsc
                )
                nc.vector.tensor_tensor(
                    sc[:, g, :], sc[:, g, :], mask[:], op=ADD
                )

            probs = smx.tile([96, 2, 96], BF16, tag="probs")
            for g in range(2):
                mx = smx.tile([96, 1], F32, tag=f"mx{g}", name=f"mx{g}")
                nc.vector.reduce_max(mx[:], sc[:, g, :], axis=AX.X)
                nmx = smx.tile([96, 1], F32, tag=f"nmx{g}", name=f"nmx{g}")
                nc.vector.tensor_scalar(nmx[:], mx[:], -1.0, None, op0=MUL)
                e = smx.tile([96, 96], F32, tag=f"e{g}", name=f"e{g}")
                nc.scalar.activation(e[:], sc[:, g, :], AF.Exp, bias=nmx[:])
                sm = smx.tile([96, 1], F32, tag=f"sm{g}", name=f"sm{g}")
                nc.vector.reduce_sum(sm[:], e[:], axis=AX.X)
                nc.vector.reciprocal(sm[:], sm[:])
                nc.vector.tensor_scalar(
                    probs[:, g, :], e[:], sm[:], None, op0=MUL
                )

            # ---------- fold M = w_proj @ probs ----------
            # mT[d,o] rows: v pblock3 channels (pair a d0:96 + pair b d0:32);
            # mT4[d,o] rows: v pblock4 channels (pair b d32:96)
            mt_psA = psB.tile([96, DIM], F32, tag="mtpsA", name="mtpsA")
            mt_psB = psB.tile([96, DIM], F32, tag="mtpsB", name="mtpsB")
            nc.tensor.matmul(
                mt_psA[:, :], probs[:, 0, :], wp[:, 0, :],
                start=True, stop=True,
            )
            nc.tensor.matmul(
                mt_psB[:, :], probs[:, 1, :], wp[:, 1, :],
                start=True, stop=True,
            )
            mT = smx.tile([128, DIM], BF16, tag="mT")
            mT4 = smx.tile([64, DIM], BF16, tag="mT4")
            mTb = smx.tile([96, DIM], BF16, tag="mTb")
            nc.scalar.copy(mT[0:96, :], mt_psA[:])
            nc.scalar.copy(mTb[:], mt_psB[:])
            nc.sync.dma_start(mT[96:128, :], mTb[0:32, :])
            nc.sync.dma_start(mT4[0:64, :], mTb[32:96, :])

        # ============ stage C: y = M @ v, streaming over hw ============
        with (
            tc.tile_pool(name="pspj", bufs=2, space=bass.MemorySpace.PSUM) as pspj,
            tc.tile_pool(name="vload", bufs=3) as vload,
            tc.tile_pool(name="outp", bufs=2) as outp,
        ):
            for jj in range(HW // 1024):
                vla = vload.tile([128, 1024], BF16, tag="vla")
                vlb = vload.tile([64, 1024], BF16, tag="vlb")
                nc.sync.dma_start(vla[:], vda[:, ts(jj, 1024)])
                nc.sync.dma_start(vlb[:], vdb[:, ts(jj, 1024)])
                ot = outp.tile([128, 2, 512], F32, tag="ot")
                ot2 = outp.tile([64, 2, 512], F32, tag="ot2")
                for u in range(2):
                    p0 = pspj.tile([128, 512], F32, tag="p0")
                    p1 = pspj.tile([64, 512], F32, tag="p1")
                    nc.tensor.matmul(
                        p0[:], mT[:, 0:128], vla[:, ts(u, 512)],
                        start=True, stop=False,
                    )
                    nc.tensor.matmul(
                        p0[:], mT4[:, 0:128], vlb[:, ts(u, 512)],
                        start=False, stop=True,
                    )
                    nc.tensor.matmul(
                        p1[:], mT[:, 128:192], vla[:, ts(u, 512)],
                        start=True, stop=False,
                    )
                    nc.tensor.matmul(
                        p1[:], mT4[:, 128:192], vlb[:, ts(u, 512)],
                        start=False, stop=True,
                    )
                    nc.scalar.copy(ot[:, u, :], p0[:])
                    nc.scalar.copy(ot2[:, u, :], p1[:])
                osl = ts(jj, 1024)
                nc.sync.dma_start(
                    outd[0:128, osl], ot[:].rearrange("p u w -> p (u w)")
                )
                nc.sync.dma_start(
                    outd[128:192, osl], ot2[:].rearrange("p u w -> p (u w)")
                )


_NC_CACHE = {}


def _get_nc():
    if "nc" not in _NC_CACHE:
        _NC_CACHE["nc"] = build()
    return _NC_CACHE["nc"]


def prep_inputs(x, w_qkv, w_dw, w_proj, temperature):
    x = np.asarray(x, np.float32)
    w_qkv = np.asarray(w_qkv, np.float32)
    w_dw = np.asarray(w_dw, np.float32).reshape(C3, 9)
    w_proj = np.asarray(w_proj, np.float32)
    temperature = np.asarray(temperature, np.float32).reshape(NH)

    wqT = np.ascontiguousarray(w_qkv.T)  # [192, 576]
    wq = np.zeros((128, 2, C3), np.float32)
    wq[:, 0, :] = wqT[0:128]
    wq[0:64, 1, :] = wqT[128:192]
    wq = wq.astype(ml_dtypes.bfloat16)

    wpT = np.ascontiguousarray(w_proj.T)  # [c, o]
    wp = np.zeros((96, 2, DIM), np.float32)
    wp[:, 0, :] = wpT[0:96]
    wp[:, 1, :] = wpT[96:192]
    wp = wp.astype(ml_dtypes.bfloat16)

    # q,k dw weights for DVE (pblocks 0..2) + v pblock4 in slot 3
    dww = np.zeros((128, 4, 9), np.float32)
    for pb in range(3):
        dww[:, pb, :] = w_dw[pb * 128 : (pb + 1) * 128]
    dww[0:64, 3, :] = w_dw[512:576]

    # v dw diag matrices for PE (pblocks 3,4)
    dvv = np.zeros((128, 9, DIM), np.float32)
    for i in range(9):
        dvv[:, i, 0:128] = np.diag(w_dw[384:512, i])
        dvv[0:64, i, 128:192] = np.diag(w_dw[512:576, i])
    dvv = dvv.astype(ml_dtypes.bfloat16)

    idshift = np.zeros((128, 64), np.float32)
    idshift[64:128, :] = np.eye(64)
    idshift = idshift.astype(ml_dtypes.bfloat16)

    mask = np.full((96, 96), -1e30, np.float32)
    mask[0:48, 0:48] = 0.0
    mask[48:96, 48:96] = 0.0

    tmpv = np.ones((128, 3), np.float32)
    tmpv[:, 0] = temperature[np.arange(128) // CH]
    tmpv[0:64, 1] = temperature[(128 + np.arange(64)) // CH]

    maps = []
    for b in range(B):
        maps.append(
            {
                "x": np.ascontiguousarray(x[b].reshape(DIM, HW)).astype(ml_dtypes.bfloat16),
                "wq": wq,
                "wp": wp,
                "dww": dww,
                "dvv": dvv,
                "tmpv": tmpv,
                "mask": mask,
                "idshift": idshift,
            }
        )
    return maps


def kernel(x, w_qkv, w_dw, w_proj, temperature, trace=False, tmpdir=None,
           **_ignored):
    nc = _get_nc()
    maps = prep_inputs(x, w_qkv, w_dw, w_proj, temperature)
    res = run_bass_kernel_spmd(
        nc, maps, core_ids=list(range(B)), trace=trace, tmpdir=tmpdir
    )
    out = np.stack(
        [np.asarray(r["out"]).reshape(DIM, H, W) for r in res.results]
    ).astype(np.float32)
    kernel.last_exec_time_ns = res.exec_time_ns
    return out


if __name__ == "__main__":
    nc = build()
    print("build ok")
